# revision 43
# baseline (speedup 1.0000x reference)
"""Causal masked attention (B=8, S=2048, d_model=1024, d_k=d_v=512) on 8 TRN2
NeuronCores, data-parallel over batch (one batch element per core).

v2 dataflow (all matmuls bf16 with fp32 PSUM accumulation):
  Host pre-casts x to bf16 and pre-arranges W into the SBUF chunk layout
  w[p, c, n] = W[128c+p, n], plus kvbias/qvalid/causal constants.
  Device loads x TRANSPOSED straight from DRAM via the DMA-xbar
  (dma_start_transpose, 16-bit path): xT tiles [dm, s] with zero PE/DVE
  cost.  No casts, no PE transposes, no PSUM staging for the front-end.
  qT = Wq^T x_q^T, kT = Wk^T x_kv^T   ([d_k, S] bf16, PE)
  v  = x_kv Wv                        ([S, d_v] bf16, PE)
  scores^T blocks [keys 128, q<=512] = kT_chunk^T @ qT, causal-skipped and
  triangular-sliced on the N (query) dim for boundary chunks.
  p^T = exp(scale*s^T + kv_bias)      (ACT, kv padding folded into bias)
  boundary blocks *= causal 0/1 tile  (DVE)
  out = p^T.T @ v, den = p^T.T @ 1    (PE), out *= qvalid/den (ACT w/ scale AP)
  The qb=15 block's accumulation is split so its last (kc=15) pair is the
  only PE work after pv(8..14), killing the tail dependency stall.

Fully-masked rows give den==0 -> clamped to 1e-30 -> out = 0 (matches the
reference's NaN->0). Query-padded rows are zeroed via qvalid.
"""

import numpy as np
import ml_dtypes
from contextlib import ExitStack

import concourse.bass as bass
import concourse.tile as tile
import concourse.mybir as mybir
from concourse import bacc
from concourse.bass_utils import run_bass_kernel_spmd

B, S, DM, DK, DV = 8, 2048, 1024, 512, 512
NCORES = 8
P = 128
NQJ = S // 512          # 4 query column-blocks of 512
NKC = S // P            # 16 key chunks of 128
NDMC = DM // P          # 8 d_model chunks
NDKC = DK // P          # 4 d_k chunks
SCALE = float(DK) ** -0.5

F32 = mybir.dt.float32
BF16 = mybir.dt.bfloat16
ts = bass.ts


def _emit(nc):
    # x arrives host-pre-transposed and bf16-pre-cast, in the SBUF chunk
    # layout xT[p, sb, c, s'] = x[512*sb + s', 128c + p].
    xq = nc.declare_dram_parameter("xq", [P, NQJ * NDMC * 512], BF16, isOutput=False)
    xkv = nc.declare_dram_parameter("xkv", [P, NQJ * NDMC * 512], BF16, isOutput=False)
    # weights pre-arranged into SBUF chunk layout w[p, c, n] = W[128c+p, n]
    wq = nc.declare_dram_parameter("wq", [P, NDMC * DK], BF16, isOutput=False)
    wk = nc.declare_dram_parameter("wk", [P, NDMC * DK], BF16, isOutput=False)
    wv = nc.declare_dram_parameter("wv", [P, NDMC * DV], BF16, isOutput=False)
    # kvbias | qvalid packed in one [P, 32] f32 tensor (single small DMA)
    kvq_d = nc.declare_dram_parameter("kvq", [P, 2 * NKC], F32, isOutput=False)
    causal_d = nc.declare_dram_parameter("causal", [P, 4 * 512], BF16, isOutput=False)
    out = nc.declare_dram_parameter("out", [S, DV], F32, isOutput=True)

    with ExitStack() as ctx:
        tc = ctx.enter_context(tile.TileContext(nc))
        cst = ctx.enter_context(tc.tile_pool(name="cst", bufs=1))
        ptp = ctx.enter_context(tc.tile_pool(name="ptp", bufs=2))
        etp = ctx.enter_context(tc.tile_pool(name="etp", bufs=2))
        obp = ctx.enter_context(tc.tile_pool(name="obp", bufs=2))
        sml = ctx.enter_context(tc.tile_pool(name="sml", bufs=4))
        psm = ctx.enter_context(tc.tile_pool(name="psm", bufs=3, space="PSUM"))
        psv = ctx.enter_context(tc.tile_pool(name="psv", bufs=3, space="PSUM"))
        psd = ctx.enter_context(tc.tile_pool(name="psd", bufs=2, space="PSUM"))

        # ---- constants (host-precomputed, tiny DMAs on scalar queue) ------
        ones = cst.tile([P, 1], BF16, tag="ones")
        nc.gpsimd.memset(ones[:], 1.0)

        kvq = cst.tile([P, 2 * NKC], F32, tag="kvq")

        def kvbias_ap(kc):
            return kvq[:, kc:kc + 1]

        def qvalid_ap(qb):
            return kvq[:, NKC + qb:NKC + qb + 1]

        causal = cst.tile([P, 4, 512], BF16, tag="causal")

        # ---- persistent projection outputs --------------------------------
        qT = cst.tile([P, NDKC, S], BF16, tag="qT")     # [dk, s]
        kT = cst.tile([P, NDKC, S], BF16, tag="kT")     # [dk, s]
        vS = cst.tile([P, NKC, DV], BF16, tag="vS")     # [s, dv]

        # ---- weights (pre-arranged bf16: w[p, c, n] = W[128c+p, n]) -------
        wqt = cst.tile([P, NDMC, DK], BF16, tag="wqt")
        wkt = cst.tile([P, NDMC, DK], BF16, tag="wkt")
        wvt = cst.tile([P, NDMC, DV], BF16, tag="wvt")

        # ---- x front-end: plain strided loads of the pre-transposed x -----
        xqT = cst.tile([P, NQJ, NDMC, 512], BF16, tag="xqT")
        xkvT = cst.tile([P, NQJ, NDMC, 512], BF16, tag="xkvT")

        def xt_load(eng, dst, xsrc, sb, c0=0, c1=NDMC):
            # dst[:, sb, c0:c1, :] <- per-sb slab of the pre-transposed x
            # (c-slices stay contiguous in DRAM: 4 KB+ descriptor rows)
            src = xsrc.rearrange("p (b c s) -> p b c s", b=NQJ, c=NDMC)
            eng.dma_start(dst[:, sb, c0:c1, :], src[:, sb, c0:c1, :])

        def _proj_cpassed(dst, wt, xt, sb, cbounds, nm):
            if len(cbounds) == 2:
                # plain: one acc at a time, copy as soon as it stops
                for d in range(NDKC):
                    acc = psm.tile([P, 512], F32, tag="mm", name=f"{nm}{sb}_{d}")
                    for c in range(NDMC):
                        nc.tensor.matmul(
                            acc[:], wt[:, c, ts(d, P)], xt[:, sb, c, :],
                            start=(c == 0), stop=(c == NDMC - 1),
                        )
                    if d % 2 == 0:
                        nc.vector.tensor_copy(dst[:, d, ts(sb, 512)], acc[:])
                    else:
                        nc.scalar.copy(dst[:, d, ts(sb, 512)], acc[:])
                return
            # c-passed start: d0/d1 accumulate the first c-half as soon as
            # it lands, then finish; d2/d3 run plain (data arrived by then).
            # Max 2 live accs so psm stays at 3 banks.
            accs = [psm.tile([P, 512], F32, tag="mm", name=f"{nm}{sb}_{d}")
                    for d in range(2)]
            for ci in range(len(cbounds) - 1):
                for d in range(2):
                    for c in range(cbounds[ci], cbounds[ci + 1]):
                        nc.tensor.matmul(
                            accs[d][:], wt[:, c, ts(d, P)], xt[:, sb, c, :],
                            start=(c == 0), stop=(c == NDMC - 1),
                        )
            nc.vector.tensor_copy(dst[:, 0, ts(sb, 512)], accs[0][:])
            nc.scalar.copy(dst[:, 1, ts(sb, 512)], accs[1][:])
            for d in range(2, NDKC):
                acc = psm.tile([P, 512], F32, tag="mm", name=f"{nm}{sb}_{d}")
                for c in range(NDMC):
                    nc.tensor.matmul(
                        acc[:], wt[:, c, ts(d, P)], xt[:, sb, c, :],
                        start=(c == 0), stop=(c == NDMC - 1),
                    )
                if d % 2 == 0:
                    nc.vector.tensor_copy(dst[:, d, ts(sb, 512)], acc[:])
                else:
                    nc.scalar.copy(dst[:, d, ts(sb, 512)], acc[:])

        def proj_q(sb, cbounds=(0, NDMC)):
            _proj_cpassed(qT, wqt, xqT, sb, cbounds, "qacc")

        def proj_kv(sb, cbounds=(0, NDMC)):
            _proj_cpassed(kT, wkt, xkvT, sb, cbounds, "kacc")
            for u in range(4):
                vacc = psm.tile([P, 512], F32, tag="mm", name=f"vacc{sb}_{u}")
                for c in range(NDMC):
                    nc.tensor.matmul(
                        vacc[:], xkvT[:, sb, c, ts(u, P)], wvt[:, c, :],
                        start=(c == 0), stop=(c == NDMC - 1),
                    )
                if u % 2 == 0:
                    nc.vector.tensor_copy(vS[:, sb * 4 + u, :], vacc[:])
                else:
                    nc.scalar.copy(vS[:, sb * 4 + u, :], vacc[:])

        pts = {}

        def scores(qj):
            nkc = 4 * qj + 4
            pt = ptp.tile([P, NKC, 512], BF16, tag="pt", name=f"pt{qj}")
            pts[qj] = pt
            for kc in range(nkc):
                r = kc - 4 * qj
                lo = 128 * r if r > 0 else 0  # triangular N-slice
                sp = psm.tile([P, 512], F32, tag="mm", name=f"sp{qj}_{kc}")
                for d in range(NDKC):
                    nc.tensor.matmul(
                        sp[:, lo:512], kT[:, d, ts(kc, P)],
                        qT[:, d, qj * 512 + lo:(qj + 1) * 512],
                        start=(d == 0), stop=(d == NDKC - 1),
                    )
                if r < 0:
                    nc.scalar.activation(
                        pt[:, kc, :], sp[:], mybir.ActivationFunctionType.Exp,
                        bias=kvbias_ap(kc), scale=SCALE,
                    )
                else:
                    et = etp.tile([P, 512], BF16, tag="et", name=f"et{qj}_{kc}")
                    nc.scalar.activation(
                        et[:, lo:512], sp[:, lo:512],
                        mybir.ActivationFunctionType.Exp,
                        bias=kvbias_ap(kc), scale=SCALE,
                    )
                    nc.vector.tensor_mul(pt[:, kc, lo:512], et[:, lo:512],
                                         causal[:, r, lo:512])

        pvs = {}

        def pv_mm(qb, kcs, start, stop):
            qj = qb // 4
            pt = pts[qj]
            if qb not in pvs:
                pvs[qb] = (
                    psv.tile([P, DV], F32, tag="pv", name=f"po{qb}"),
                    psd.tile([P, 1], F32, tag="pd", name=f"pd{qb}"),
                )
            po, pd = pvs[qb]
            last = kcs[-1]
            for kc in kcs:
                lhs = pt[:, kc, ts(qb % 4, P)]
                nc.tensor.matmul(po[:], lhs, vS[:, kc, :],
                                 start=(start and kc == kcs[0]),
                                 stop=(stop and kc == last))
                nc.tensor.matmul(pd[:], lhs, ones[:],
                                 start=(start and kc == kcs[0]),
                                 stop=(stop and kc == last))

        def pv_fin(qb):
            po, pd = pvs[qb]
            den = sml.tile([P, 1], F32, tag="den_s", name=f"den{qb}")
            nc.vector.tensor_scalar_max(den[:], pd[:], 1e-30)
            rec = sml.tile([P, 1], F32, tag="rec", name=f"rec{qb}")
            nc.vector.reciprocal(rec[:], den[:])
            sc = sml.tile([P, 1], F32, tag="sc", name=f"sc{qb}")
            nc.vector.tensor_scalar_mul(sc[:], rec[:], qvalid_ap(qb))
            ob = obp.tile([P, DV], F32, tag="ob", name=f"ob{qb}")
            if qb == 15:
                # split the last block so its store starts earlier
                nc.vector.tensor_scalar_mul(ob[:, 0:256], po[:, 0:256], sc[:])
                nc.scalar.dma_start(out[ts(qb, P), 0:256], ob[:, 0:256])
                nc.scalar.mul(ob[:, 256:512], po[:, 256:512], sc[:])
                nc.sync.dma_start(out[ts(qb, P), 256:512], ob[:, 256:512])
                return
            if qb % 2 == 0:
                nc.scalar.mul(ob[:], po[:], sc[:])
            else:
                nc.vector.tensor_scalar_mul(ob[:], po[:], sc[:])
            nc.scalar.dma_start(out[ts(qb, P), :], ob[:])

        def pv(qb):
            pv_mm(qb, list(range(qb + 1)), True, True)
            pv_fin(qb)

        # ---- schedule -----------------------------------------------------
        # Critical path pairs the two queues; the first wq/xq0 halves are
        # 512 KB each so the PE starts during the cold-DMA ramp.
        nc.scalar.dma_start(wqt[:, 0:4, :], wq[:, 0:4 * DK])
        xt_load(nc.sync, xqT, xq, 0, 0, 4)
        nc.scalar.dma_start(wqt[:, 4:8, :], wq[:, 4 * DK:8 * DK])
        xt_load(nc.sync, xqT, xq, 0, 4, 8)
        xt_load(nc.scalar, xkvT, xkv, 0)
        nc.sync.dma_start(wkt[:], wk[:, :])
        nc.scalar.dma_start(kvq[:], kvq_d[:, :])
        nc.sync.dma_start(wvt[:], wv[:, :])
        nc.scalar.dma_start(causal[:], causal_d[:, :])
        xt_load(nc.sync, xqT, xq, 1)
        xt_load(nc.scalar, xkvT, xkv, 1)
        xt_load(nc.sync, xqT, xq, 2)
        xt_load(nc.scalar, xkvT, xkv, 2)
        xt_load(nc.sync, xqT, xq, 3)
        xt_load(nc.scalar, xkvT, xkv, 3)

        proj_q(0, cbounds=(0, 4, 8))
        proj_kv(0); scores(0)
        proj_q(1)
        proj_kv(1); scores(1)
        for qb in range(0, 4):
            pv(qb)
        proj_q(2)
        proj_kv(2); scores(2)
        for qb in range(4, 8):
            pv(qb)
        proj_q(3)
        proj_kv(3); scores(3)
        for qb in range(8, 16):
            pv(qb)

    nc.compile()
    return nc


_NC_CACHE = []


def _get_nc():
    if not _NC_CACHE:
        nc = bacc.Bacc("TRN2")
        _NC_CACHE.append(_emit(nc))
    return _NC_CACHE[0]


def _prep_w(W):
    wb = np.asarray(W, dtype=np.float32).astype(ml_dtypes.bfloat16)
    return np.ascontiguousarray(
        wb.reshape(NDMC, P, -1).transpose(1, 0, 2).reshape(P, -1))


def _prep_w_dmajor(W):
    wb = np.asarray(W, dtype=np.float32).astype(ml_dtypes.bfloat16)
    return np.ascontiguousarray(
        wb.reshape(NDMC, P, NDKC, P).transpose(1, 2, 0, 3).reshape(P, -1))


_CONST_CACHE = {}


def _causal_const():
    if "causal" not in _CONST_CACHE:
        k = np.arange(P)[:, None, None]
        r = np.arange(4)[None, :, None]
        q = np.arange(512)[None, None, :]
        c = (q >= k + 128 * r).astype(ml_dtypes.bfloat16)
        _CONST_CACHE["causal"] = np.ascontiguousarray(c.reshape(P, 4 * 512))
    return _CONST_CACHE["causal"]


def _prep_x(x_b):
    # [S, DM] -> xT[p, sb, c, s'] = x[512*sb + s', 128c + p]
    return np.ascontiguousarray(
        x_b.reshape(NQJ, 512, NDMC, P).transpose(3, 0, 2, 1).reshape(P, -1))


def _in_maps(inputs):
    sq = np.asarray(inputs["source_query"], dtype=np.float32)
    skv = np.asarray(inputs["source_key_value"], dtype=np.float32)
    qp = np.asarray(inputs["source_query_padding_mask"])
    kvp = np.asarray(inputs["source_key_value_padding_mask"])
    sq_b = sq.astype(ml_dtypes.bfloat16)
    skv_b = skv.astype(ml_dtypes.bfloat16)
    wq_a = _prep_w(inputs["Wq"])
    wk_a = _prep_w(inputs["Wk"])
    wv_a = _prep_w(inputs["Wv"])
    causal = _causal_const()
    maps = []
    for b in range(NCORES):
        kvbias = kvp[b].reshape(NKC, P).T.astype(np.float32) * np.float32(-1e9)
        qvalid = 1.0 - qp[b].reshape(NKC, P).T.astype(np.float32)
        kvq = np.ascontiguousarray(np.concatenate([kvbias, qvalid], axis=1))
        maps.append({
            "xq": _prep_x(sq_b[b]),
            "xkv": _prep_x(skv_b[b]),
            "wq": wq_a, "wk": wk_a, "wv": wv_a,
            "kvq": kvq, "causal": causal,
        })
    return maps


def _execute(inputs, **kw):
    nc = _get_nc()
    res = run_bass_kernel_spmd(nc, _in_maps(inputs), core_ids=list(range(NCORES)), **kw)
    outs = np.stack([res.results[b]["out"] for b in range(NCORES)], axis=0)
    return outs.astype(np.float32), res


def kernel(**inputs) -> np.ndarray:
    out, _ = _execute(inputs)
    return out
